# revision 2
# baseline (speedup 1.0000x reference)
"""Trainium2 Bass kernel for nn_CholecFixScore (pairwise-IoU mask scoring).

Math (per sample n):
    Gp (P=16, HW) and Gt (T=8, HW) are binary {0,1} masks.
    inters[p,t] = sum_hw Gp[p]*Gt[t];  sp[p] = sum Gp[p];  st[t] = sum Gt[t]
    iou = inters / max(sp+st-inters, 1)            (union==0 => inters==0 => iou 0)
    w[p] = max_t iou[p,t]
    den[hw] = sum_p Gp[p,hw];  r = 1/max(den,1)    (den==0 pixels have Gp==0)
    score[n] = (1/HW) * sum_p w[p] * S[p],  S[p] = sum_hw Gp[p,hw]*r[hw]
which equals the reference's mean over pixels of (sum_p w[p]Gp[p,hw])/den[hw].

Sharding: pure data parallel, 2 samples per core on 8 cores.

Precision: masks are {0,1} so bf16 operands are exact and all PE sums
accumulate exactly in fp32 PSUM.  The only real-valued rhs, r = 1/den,
is shipped as an exact two-term bf16 split (r = r_hi + r_lo + O(2^-17)),
giving two extra rhs columns whose partial sums are re-added in fp32.

On-chip layout: pixel index hw = part*392 + j  (part=0..127, j=0..391).
    Gp_sb  (128, 16*392) bf16  free = (p, j)      [SWDGE cast DMA; den chain]
    Gp_w   (128, 16*392) bf16  free = (c, js, p)  [weight layout, ScalarE shuffle]
    Gt_ext (128, 11*392) bf16  free = (u, j), u = 8 Gt | ones | r_hi | r_lo
Main pass: 49 accumulating bf16 matmuls; chunk c contracts the 128 partitions
for j in [8c, 8c+8): lhsT = Gp_w[:, 128c:+128] (M = js*16+p), rhs = Gt_ext
slice (N = js'*11+u = 88).  Valid outputs live on the js==js' block diagonal
of the (128, 88) PSUM tile; 8 selector matmuls against eye(128) columns
relocate+sum the blocks into a (16, 11) fp32 accumulator
[inters | sp | S_hi | S_lo].
"""

import numpy as np

import concourse.bass as bass
import concourse.tile as tile
from concourse import mybir
from concourse.bass_utils import run_bass_kernel_spmd

F32 = mybir.dt.float32
BF16 = mybir.dt.bfloat16
ADD = mybir.AluOpType.add

N, P, T = 16, 16, 8
H, W = 224, 224
HW = H * W            # 50176
PART = 128
JW = HW // PART       # 392 columns per mask
J = 8                 # j values batched per main-pass matmul chunk
NCH = JW // J         # 49 main-pass chunks
J_ST = 49             # j values per st-pass matmul (N = 49*8 = 392)
NCH_ST = JW // J_ST   # 8 st-pass chunks
U = T + 3             # rhs column groups: 8 Gt | ones | r_hi | r_lo
ONES_C = T * JW       # col offset of ones region in Gt_ext
RHI_C = (T + 1) * JW
RLO_C = (T + 2) * JW
NCORES = 8
SPC = N // NCORES     # samples per core = 2
INV_HW = 1.0 / HW
GP_CH = 2             # masks per Gp DMA chunk (8 chunks/sample, ~0.4 MB each)
GT_CH = 2             # masks per Gt DMA chunk (4 chunks/sample)


def _split_multi_waits(nc):
    """The pinned walrus encodes only ONE sync-wait per instruction; split
    Tile-emitted multi-wait instructions into single-wait NOPs ahead of them
    (same engine, program order => identical semantics)."""
    n = 0
    for f in nc.m.functions:
        for bb in f.blocks:
            insts = bb.instructions
            newlist = []
            changed = False
            for ins in insts:
                si = ins.sync_info
                if si is not None and si.on_wait is not None and len(si.on_wait) > 1:
                    waits = list(si.on_wait)
                    for w in waits[:-1]:
                        n += 1
                        newlist.append(
                            mybir.InstNoOp(
                                name=f"I-waitsplit-{n}",
                                engine=ins.engine,
                                ins=[],
                                outs=[],
                                sync_info=mybir.SyncInfo(on_wait=[w], on_update=[]),
                            )
                        )
                    ins.sync_info = mybir.SyncInfo(
                        on_wait=[waits[-1]], on_update=list(si.on_update or [])
                    )
                    changed = True
                newlist.append(ins)
            if changed:
                while len(insts):
                    insts.pop()
                for x in newlist:
                    insts.append(x)
    return n


def _build():
    nc = bass.Bass("TRN2", target_bir_lowering=False, debug=False)
    gp = nc.dram_tensor("gp", [SPC, P, PART, JW], F32, kind="ExternalInput")
    gt = nc.dram_tensor("gt", [SPC, T, PART, JW], F32, kind="ExternalInput")
    ce = nc.dram_tensor("ce", [PART, PART], F32, kind="ExternalInput")  # eye(128)
    y = nc.dram_tensor("y", [1, SPC], F32, kind="ExternalOutput")

    with tile.TileContext(nc) as tc:
        with (
            tc.tile_pool(name="big", bufs=2) as big,
            tc.tile_pool(name="scratch", bufs=1) as scratch,
            tc.tile_pool(name="small", bufs=2) as small,
            tc.tile_pool(name="singles", bufs=1) as singles,
            tc.tile_pool(name="psmain", bufs=2, space="PSUM") as psmain,
            tc.tile_pool(name="psaux", bufs=1, space="PSUM") as psaux,
        ):
            e_sb = singles.tile([PART, PART], F32)
            out_sb = singles.tile([1, SPC], F32)

            gps, gts, gpws = [], [], []
            for s in range(SPC):
                gps.append(big.tile([PART, P * JW], BF16, tag="gp", name=f"gp_sb{s}"))
                gts.append(big.tile([PART, U * JW], BF16, tag="gt", name=f"gt_sb{s}"))
                gpws.append(big.tile([PART, P * JW], BF16, tag="gpw", name=f"gp_w{s}"))

            # ---- input DMAs first (0.4 MB chunks, SWDGE fp32->bf16 cast).
            # Gp is chunked by mask pair (feeds the den pair-adds); Gt is
            # chunked by j-range so st/main matmuls can stream behind it. ----
            def dma_gt(s, lo, hi):
                src = gt[s, :, :, lo:hi].rearrange("t part j -> part t j")
                dst = gts[s][:].rearrange("part (u j) -> part u j", j=JW)[
                    :, 0:T, lo:hi
                ]
                nc.gpsimd.dma_start(out=dst, in_=src)

            def dma_gp(s, lo, hi):
                src = gp[s, lo:hi, :, :].rearrange("p part j -> part p j")
                dst = gps[s][:].rearrange("part (p j) -> part p j", j=JW)[:, lo:hi, :]
                nc.gpsimd.dma_start(out=dst, in_=src)

            ones16f = singles.tile([1, 16], F32)
            ones16c = singles.tile([16, 1], F32)
            with tc.high_priority():
                for s in range(SPC):
                    nc.gpsimd.memset(gts[s][:, ONES_C : ONES_C + JW], 1.0)
                nc.gpsimd.memset(ones16f[:, :], 1.0)
                nc.gpsimd.memset(ones16c[:, :], 1.0)
                nc.sync.dma_start(out=e_sb[:, :], in_=ce[:, :])

            for lo in range(0, JW, JW // 2):
                dma_gt(0, lo, lo + JW // 2)
            for lo in range(0, P, GP_CH):
                dma_gp(0, lo, lo + GP_CH)
            for lo in range(0, P, GP_CH):
                dma_gp(1, lo, lo + GP_CH)
            for lo in range(0, JW, JW // 2):
                dma_gt(1, lo, lo + JW // 2)

            # ---- per-sample pipelines ----
            gt_vs, ps_sts, accs = {}, {}, {}

            def st_pass(s):
                # st partials: ps_st[0, (js', t)] += ones^T @ Gt   (PE, bf16)
                gt_sb = gts[s]
                ones_col = gt_sb[:, ONES_C : ONES_C + 1]
                ps_st = psaux.tile([1, J_ST * T], F32, tag=f"st{s}", name=f"ps_st{s}")
                gt_v = gt_sb[:].rearrange("part (u j) -> part j u", j=JW)
                gt_vs[s], ps_sts[s] = gt_v, ps_st
                for c in range(NCH_ST):
                    nc.tensor.matmul(
                        ps_st[:, :],
                        ones_col,
                        gt_v[:, c * J_ST : (c + 1) * J_ST, 0:T],
                        start=(c == 0),
                        stop=(c == NCH_ST - 1),
                    )

            def den_chain(s):
                # den = sum_p Gp[p]: independent 2-mask pair sums (one per DMA
                # chunk) then a short accumulation chain; r = 1/max(den,1) and
                # an exact bf16 split r = r_hi + r_lo into the rhs extension.
                gp_sb, gt_sb = gps[s], gts[s]
                pairs = scratch.tile(
                    [PART, (P // 2) * JW], BF16, tag="pairs", name=f"pairs{s}"
                )
                den = scratch.tile([PART, JW], BF16, tag="den", name=f"den{s}")
                with tc.high_priority():
                    for i in range(P // 2):
                        nc.vector.tensor_tensor(
                            pairs[:, i * JW : (i + 1) * JW],
                            gp_sb[:, 2 * i * JW : (2 * i + 1) * JW],
                            gp_sb[:, (2 * i + 1) * JW : (2 * i + 2) * JW],
                            ADD,
                        )
                    nc.vector.tensor_tensor(
                        den[:], pairs[:, 0:JW], pairs[:, JW : 2 * JW], ADD
                    )
                    for i in range(2, P // 2):
                        nc.vector.tensor_tensor(
                            den[:], den[:], pairs[:, i * JW : (i + 1) * JW], ADD
                        )
                    nc.vector.tensor_scalar_max(out=den[:], in0=den[:], scalar1=1.0)
                    r32 = scratch.tile([PART, JW], F32, tag="r32", name=f"r32_{s}")
                    nc.vector.reciprocal(out=r32[:], in_=den[:])
                    rhi = gt_sb[:, RHI_C : RHI_C + JW]
                    nc.vector.tensor_copy(rhi, r32[:])
                    nc.vector.tensor_tensor(
                        gt_sb[:, RLO_C : RLO_C + JW], r32[:], rhi,
                        mybir.AluOpType.subtract,
                    )

            def shuffles(s):
                # weight-layout shuffle on ScalarE, one copy per 2-mask chunk
                gp_sb, gp_w = gps[s], gpws[s]
                wv = gp_w[:].rearrange("part (c js p) -> part c js p", js=J, p=P)
                sv = gp_sb[:].rearrange("part (p c js) -> part c js p", p=P, js=J)
                for lo in range(0, P, GP_CH):
                    nc.scalar.copy(
                        out=wv[:, :, :, lo : lo + GP_CH],
                        in_=sv[:, :, :, lo : lo + GP_CH],
                    )

            def main_pass(s):
                gp_w, gt_v = gpws[s], gt_vs[s]
                ps_main = psmain.tile(
                    [PART, J * U], F32, tag="main", name=f"ps_main{s}"
                )
                for c in range(NCH):
                    nc.tensor.matmul(
                        ps_main[:, :],
                        gp_w[:, c * PART : (c + 1) * PART],
                        gt_v[:, c * J : (c + 1) * J, :],
                        start=(c == 0),
                        stop=(c == NCH - 1),
                    )
                # extraction: sum the 8 diagonal (16, U) blocks via eye cols
                ext = small.tile([PART, J * U], F32, tag="ext", name=f"ext{s}")
                nc.vector.tensor_copy(ext[:, :], ps_main[:, :])
                ps_acc = psaux.tile([16, U], F32, tag="acc", name=f"ps_acc{s}")
                for js in range(J):
                    nc.tensor.matmul(
                        ps_acc[:, :],
                        e_sb[:, js * 16 : (js + 1) * 16],
                        ext[:, js * U : (js + 1) * U],
                        start=(js == 0),
                        stop=(js == J - 1),
                    )
                acc = small.tile([16, U], F32, tag=f"accsb{s}", name=f"acc{s}")
                nc.vector.tensor_copy(acc[:, :], ps_acc[:, :])
                accs[s] = acc

            def finish(s):
                ps_st, acc = ps_sts[s], accs[s]
                # st: reduce partials, broadcast to 16 partitions via tiny mm
                st_sb = small.tile([1, T], F32, tag=f"stsb{s}", name=f"st_sb{s}")
                nc.vector.tensor_reduce(
                    out=st_sb[:, :],
                    in_=ps_st[:].rearrange("p (j t) -> p t j", t=T),
                    axis=mybir.AxisListType.X,
                    op=ADD,
                )
                ps_st16 = psaux.tile([16, T], F32, tag="st16", name=f"ps_st16{s}")
                nc.tensor.matmul(ps_st16[:, :], ones16f[:, :], st_sb[:, :])
                # unions = max((st16 + sp) - inters, 1);  iou = inters/unions
                unions = small.tile([16, T], F32, tag=f"un{s}", name=f"unions{s}")
                nc.vector.scalar_tensor_tensor(
                    out=unions[:, :],
                    in0=ps_st16[:, :],
                    scalar=acc[:, T : T + 1],
                    in1=acc[:, 0:T],
                    op0=ADD,
                    op1=mybir.AluOpType.subtract,
                )
                nc.vector.tensor_scalar_max(
                    out=unions[:, :], in0=unions[:, :], scalar1=1.0
                )
                nc.vector.reciprocal(out=unions[:, :], in_=unions[:, :])
                iou = small.tile([16, T], F32, tag=f"iou{s}", name=f"iou{s}")
                nc.vector.tensor_tensor(
                    iou[:, :], acc[:, 0:T], unions[:, :], mybir.AluOpType.mult
                )
                wmax = small.tile([16, 1], F32, tag=f"wm{s}", name=f"wmax{s}")
                nc.vector.tensor_reduce(
                    out=wmax[:, :],
                    in_=iou[:, :],
                    axis=mybir.AxisListType.X,
                    op=mybir.AluOpType.max,
                )
                # ws = (S_hi + S_lo) * w
                ws = small.tile([16, 1], F32, tag=f"ws{s}", name=f"ws{s}")
                nc.vector.scalar_tensor_tensor(
                    out=ws[:, :],
                    in0=acc[:, T + 1 : T + 2],
                    scalar=acc[:, T + 2 : T + 3],
                    in1=wmax[:, :],
                    op0=ADD,
                    op1=mybir.AluOpType.mult,
                )
                ps_score = psaux.tile([1, 1], F32, tag="sc", name=f"ps_score{s}")
                nc.tensor.matmul(ps_score[:, :], ones16c[:, :], ws[:, :])
                nc.vector.tensor_scalar_mul(
                    out=out_sb[0:1, s : s + 1], in0=ps_score[:, :], scalar1=INV_HW
                )

            # trace order chosen so each engine's FIFO matches data arrival:
            # PE: st0, st1, main0, main1.  DVE: den0, den1, then epilogues.
            st_pass(0)
            den_chain(0)
            shuffles(0)
            den_chain(1)
            shuffles(1)
            main_pass(0)
            finish(0)
            st_pass(1)
            main_pass(1)
            finish(1)

            nc.sync.dma_start(out=y[:, :], in_=out_sb[:, :])

    _split_multi_waits(nc)
    return nc


_NC = None


def _get_nc():
    global _NC
    if _NC is None:
        _NC = _build()
    return _NC


def make_in_maps(groups_pred: np.ndarray, groups_true: np.ndarray):
    gp = np.ascontiguousarray(groups_pred, dtype=np.float32).reshape(
        NCORES, SPC, P, PART, JW
    )
    gt = np.ascontiguousarray(groups_true, dtype=np.float32).reshape(
        NCORES, SPC, T, PART, JW
    )
    ce = np.eye(PART, dtype=np.float32)
    return [{"gp": gp[c], "gt": gt[c], "ce": ce} for c in range(NCORES)]


def kernel(groups_pred: np.ndarray, groups_true: np.ndarray) -> np.ndarray:
    assert groups_pred.shape == (N, P, H, W)
    assert groups_true.shape == (N, T, H, W)
    in_maps = make_in_maps(groups_pred, groups_true)
    res = run_bass_kernel_spmd(_get_nc(), in_maps, core_ids=list(range(NCORES)))
    out = np.empty((N,), dtype=np.float32)
    for c in range(NCORES):
        out[c * SPC : (c + 1) * SPC] = res.results[c]["y"][0]
    return out



# revision 5
# speedup vs baseline: 1.1418x; 1.1418x over previous
"""Trainium2 Bass kernel for nn_CholecFixScore (pairwise-IoU mask scoring).

Math (per sample n):
    Gp (P=16, HW) and Gt (T=8, HW) are binary {0,1} masks.
    inters[p,t] = sum_hw Gp[p]*Gt[t];  sp[p] = sum Gp[p];  st[t] = sum Gt[t]
    iou = inters / max(sp+st-inters, 1)            (union==0 => inters==0 => iou 0)
    w[p] = max_t iou[p,t]
    den[hw] = sum_p Gp[p,hw];  r = 1/max(den,1)    (den==0 pixels have Gp==0)
    score[n] = (1/HW) * sum_p w[p] * S[p],  S[p] = sum_hw Gp[p,hw]*r[hw]
which equals the reference's mean over pixels of (sum_p w[p]Gp[p,hw])/den[hw].

Sharding: pure data parallel, 2 samples per core on 8 cores.

On-chip layout (hw = part*392 + j, part=0..127, j=0..391), all host-staged so
every DMA is contiguous and every matmul rhs is stride-1:
    gpw_sb (128, 392*16) bf16, free=(j, p)      [SWDGE cast DMA from (part,j,p) f32]
    gtd_sb (128, 392*9)  bf16, free=(j, t), t=8 is a host-staged ones plane
    r2     (128, 392*2)  bf16, free=(j, {r_hi, r_lo}), written after den

Passes per sample:
    st:     7 PE matmuls, lhsT=ones column, rhs=gtd 504-col groups, PSUM-
            accumulated partials (j mod 56) -> DVE reduce -> st[t] (t=8 ones
            slot doubles as a pixel count, unused).
    den:    one DVE reduce over the innermost p axis of gpw_sb; sums <=16 are
            exact in bf16.  r=1/max(den,1) split exactly as r_hi+r_lo bf16.
    inters+sp: 49 accumulating matmuls, lhsT=gpw chunk (128 cols = 8j x 16p),
            rhs=gtd chunk (72 contiguous cols = 8j x 9t).  Valid outputs on
            the j-diagonal (16,9) blocks; t=8 col gives sp.
    S:      49 matmuls, same lhsT chunks, rhs=r2 chunk (16 cols).
    extract: PSUM->SBUF bf16 copy, then 8 eye-selector matmuls per PSUM tile
            relocate+sum the diagonal blocks into (16,9)/(16,2) accumulators.
    epilogue: unions/iou/wmax/ws on DVE, two tiny fp32 matmuls, scale, DMA out.

Precision: masks are {0,1} so bf16 operands are exact and PE sums accumulate
exactly in fp32 PSUM.  extM/extS round the fp32 partials to bf16 once
(rel err ~2^-9) before the selector sums - well inside the 2e-2 gate.
"""

import numpy as np
import ml_dtypes

import concourse.bass as bass
import concourse.tile as tile
from concourse import mybir
from concourse.bass_utils import run_bass_kernel_spmd

F32 = mybir.dt.float32
BF16 = mybir.dt.bfloat16
ADD = mybir.AluOpType.add
SUB = mybir.AluOpType.subtract
MULT = mybir.AluOpType.mult
MAXOP = mybir.AluOpType.max

N, P, T = 16, 16, 8
H, W = 224, 224
HW = H * W            # 50176
PART = 128
JW = HW // PART       # 392 pixel columns per partition
U = T + 1             # 8 Gt planes + host-staged ones plane (-> sp)
J = 8                 # j values per main-pass chunk (M = 8*16 = 128)
NCH = JW // J         # 49 main-pass chunks
J_ST = 56             # j values per st matmul group (N = 56*9 = 504)
NST = JW // J_ST      # 7 st groups
NCORES = 8
SPC = N // NCORES     # samples per core
INV_HW = 1.0 / HW
NQ = 4                # gtd1 arrives in NQ c-aligned chunks for tail pipelining
# c-chunk boundaries per quarter: 12/12/12/13
QC = [(0, 12), (12, 24), (24, 36), (36, 49)]


def _split_multi_waits(nc):
    """The pinned walrus encodes only ONE sync-wait per instruction; split
    Tile-emitted multi-wait instructions into single-wait NOPs ahead of them
    (same engine, program order => identical semantics)."""
    n = 0
    for f in nc.m.functions:
        for bb in f.blocks:
            insts = bb.instructions
            newlist = []
            changed = False
            for ins in insts:
                si = ins.sync_info
                if si is not None and si.on_wait is not None and len(si.on_wait) > 1:
                    waits = list(si.on_wait)
                    for w in waits[:-1]:
                        n += 1
                        newlist.append(
                            mybir.InstNoOp(
                                name=f"I-waitsplit-{n}",
                                engine=ins.engine,
                                ins=[],
                                outs=[],
                                sync_info=mybir.SyncInfo(on_wait=[w], on_update=[]),
                            )
                        )
                    ins.sync_info = mybir.SyncInfo(
                        on_wait=[waits[-1]], on_update=list(si.on_update or [])
                    )
                    changed = True
                newlist.append(ins)
            if changed:
                while len(insts):
                    insts.pop()
                for x in newlist:
                    insts.append(x)
    return n


def _build():
    nc = bass.Bass("TRN2", target_bir_lowering=False, debug=False)
    gpw = nc.dram_tensor("gpw", [SPC, PART, JW, P], F32, kind="ExternalInput")
    gtd = nc.dram_tensor("gtd", [SPC, PART, JW, U], F32, kind="ExternalInput")
    cab = nc.dram_tensor("cab", [PART, PART + 1], BF16, kind="ExternalInput")
    aux = nc.dram_tensor("aux", [16, 17], F32, kind="ExternalInput")
    y = nc.dram_tensor("y", [1, SPC], F32, kind="ExternalOutput")

    with tile.TileContext(nc) as tc:
        with (
            tc.tile_pool(name="big", bufs=2) as big,
            tc.tile_pool(name="scratch", bufs=1) as scratch,
            tc.tile_pool(name="small", bufs=2) as small,
            tc.tile_pool(name="singles", bufs=1) as singles,
            tc.tile_pool(name="psm", bufs=1, space="PSUM") as psm,
            tc.tile_pool(name="pss", bufs=1, space="PSUM") as pss,
            tc.tile_pool(name="psst", bufs=1, space="PSUM") as psst,
            tc.tile_pool(name="psaux", bufs=1, space="PSUM") as psaux,
        ):
            cab_sb = singles.tile([PART, PART + 1], BF16)
            aux_sb = singles.tile([16, 17], F32)
            out_sb = singles.tile([1, SPC], F32)
            ones_col = cab_sb[:, PART : PART + 1]
            ones16f = aux_sb[0:1, 1:17]
            ones16c = aux_sb[0:16, 0:1]

            gpws, gtds, r2s = [], [], []
            for s in range(SPC):
                gpws.append(big.tile([PART, JW * P], BF16, tag="gpw", name=f"gpw{s}"))
                gtds.append(big.tile([PART, JW * U], BF16, tag="gtd", name=f"gtd{s}"))
                r2s.append(big.tile([PART, JW * 2], BF16, tag="r2", name=f"r2_{s}"))

            # ---- DMAs: 2 tiny HWDGE consts + 7 big SWDGE cast transfers ----
            with tc.high_priority():
                nc.sync.dma_start(out=cab_sb[:, :], in_=cab[:, :])
                nc.sync.dma_start(out=aux_sb[:, :], in_=aux[:, :])
                gtd0_v = gtds[0][:].rearrange("part (j t) -> part j t", t=U)
                nc.gpsimd.dma_start(out=gtd0_v, in_=gtd[0])
                for s in range(SPC):
                    gv = gpws[s][:].rearrange("part (j p) -> part j p", p=P)
                    nc.gpsimd.dma_start(out=gv, in_=gpw[s])
                gtd1_v = gtds[1][:].rearrange("part (j t) -> part j t", t=U)
                for c0, c1 in QC:
                    nc.gpsimd.dma_start(
                        out=gtd1_v[:, c0 * J : c1 * J, :],
                        in_=gtd[1, :, c0 * J : c1 * J, :],
                    )

            # ---- per-sample pieces ----
            stps, stsbs, accs, st16s, wss, scs = {}, {}, {}, {}, {}, {}

            def st_mms(s, g0, g1):
                # st partials: ps_stp[0, (jl, t)] += ones^T @ Gtd group (PE)
                if s not in stps:
                    stps[s] = psst.tile([1, J_ST * U], F32, tag="stp", name=f"stp{s}")
                gv = gtds[s][:]
                for g in range(g0, g1):
                    nc.tensor.matmul(
                        stps[s][:, :],
                        ones_col,
                        gv[:, g * J_ST * U : (g + 1) * J_ST * U],
                        start=(g == 0),
                        stop=(g == NST - 1),
                    )

            def st_reduce(s):
                stsb = small.tile([1, U], F32, tag="stsb", name=f"stsb{s}")
                nc.vector.tensor_reduce(
                    out=stsb[:, :],
                    in_=stps[s][:].rearrange("p (jl t) -> p t jl", t=U),
                    axis=mybir.AxisListType.X,
                    op=ADD,
                )
                stsbs[s] = stsb

            def st16_mm(s):
                ps = psaux.tile([16, T], F32, tag="st16", name=f"st16_{s}")
                nc.tensor.matmul(ps[:, :], ones16f, stsbs[s][:, 0:T])
                st16s[s] = ps

            def den_r(s):
                den = scratch.tile([PART, JW], BF16, tag="den", name=f"den{s}")
                with nc.allow_low_precision("binary mask sums <= 16 exact in bf16"):
                    nc.vector.tensor_reduce(
                        out=den[:, :],
                        in_=gpws[s][:].rearrange("part (j p) -> part j p", p=P),
                        axis=mybir.AxisListType.X,
                        op=ADD,
                    )
                nc.vector.tensor_scalar_max(out=den[:, :], in0=den[:, :], scalar1=1.0)
                r32 = scratch.tile([PART, JW], F32, tag="r32", name=f"r32_{s}")
                nc.vector.reciprocal(out=r32[:, :], in_=den[:, :])
                rv = r2s[s][:].rearrange("part (j two) -> part j two", two=2)
                nc.vector.tensor_copy(rv[:, :, 0], r32[:, :])
                nc.vector.tensor_tensor(rv[:, :, 1], r32[:, :], rv[:, :, 0], SUB)

            def get_ps(s):
                if ("m", s) not in accs:
                    accs[("m", s)] = psm.tile(
                        [PART, J * U], F32, tag="m", name=f"psm{s}"
                    )
                    accs[("s", s)] = pss.tile(
                        [PART, J * 2], F32, tag="s", name=f"pss{s}"
                    )
                return accs[("m", s)], accs[("s", s)]

            def inters_s_mms(s, c0, c1, with_s):
                # inters+sp (N=72) and S (N=16) accumulating matmuls, shared lhsT
                ps_m, ps_s = get_ps(s)
                gp_sb, gt_sb, r2 = gpws[s], gtds[s], r2s[s]
                for c in range(c0, c1):
                    lhsT = gp_sb[:, c * PART : (c + 1) * PART]
                    nc.tensor.matmul(
                        ps_m[:, :],
                        lhsT,
                        gt_sb[:, c * J * U : (c + 1) * J * U],
                        start=(c == 0),
                        stop=(c == NCH - 1),
                    )
                    if with_s:
                        nc.tensor.matmul(
                            ps_s[:, :],
                            lhsT,
                            r2[:, c * J * 2 : (c + 1) * J * 2],
                            start=(c == 0),
                            stop=(c == NCH - 1),
                        )

            def extract_copies(s):
                extM = small.tile([PART, J * U], BF16, tag="extM", name=f"extM{s}")
                extS = small.tile([PART, J * 2], BF16, tag="extS", name=f"extS{s}")
                nc.scalar.copy(out=extM[:, :], in_=accs[("m", s)][:, :])
                nc.scalar.copy(out=extS[:, :], in_=accs[("s", s)][:, :])
                return extM, extS

            def selectors(s, extM, extS):
                ps_a = psaux.tile([16, U], F32, tag="acc", name=f"ps_acc{s}")
                ps_b = psaux.tile([16, 2], F32, tag="accS", name=f"ps_accS{s}")
                for js in range(J):
                    nc.tensor.matmul(
                        ps_a[:, :],
                        cab_sb[:, js * 16 : (js + 1) * 16],
                        extM[:, js * U : (js + 1) * U],
                        start=(js == 0),
                        stop=(js == J - 1),
                    )
                for js in range(J):
                    nc.tensor.matmul(
                        ps_b[:, :],
                        cab_sb[:, js * 16 : (js + 1) * 16],
                        extS[:, js * 2 : (js + 1) * 2],
                        start=(js == 0),
                        stop=(js == J - 1),
                    )
                return ps_a, ps_b

            def epilogue_v(s, ps_a, ps_b):
                # acc: [inters(8) | sp | S_hi | S_lo]
                acc = small.tile([16, U + 2], F32, tag="acc", name=f"acc{s}")
                nc.vector.tensor_copy(acc[:, 0:U], ps_a[:, :])
                nc.vector.tensor_copy(acc[:, U : U + 2], ps_b[:, :])
                unions = small.tile([16, T], F32, tag="un", name=f"un{s}")
                nc.vector.scalar_tensor_tensor(
                    out=unions[:, :],
                    in0=st16s[s][:, :],
                    scalar=acc[:, T : T + 1],
                    in1=acc[:, 0:T],
                    op0=ADD,
                    op1=SUB,
                )
                nc.vector.tensor_scalar_max(
                    out=unions[:, :], in0=unions[:, :], scalar1=1.0
                )
                nc.vector.reciprocal(out=unions[:, :], in_=unions[:, :])
                iou = small.tile([16, T], F32, tag="iou", name=f"iou{s}")
                nc.vector.tensor_tensor(iou[:, :], acc[:, 0:T], unions[:, :], MULT)
                wmax = small.tile([16, 1], F32, tag="wm", name=f"wm{s}")
                nc.vector.tensor_reduce(
                    out=wmax[:, :], in_=iou[:, :], axis=mybir.AxisListType.X, op=MAXOP
                )
                ws = small.tile([16, 1], F32, tag="ws", name=f"ws{s}")
                nc.vector.scalar_tensor_tensor(
                    out=ws[:, :],
                    in0=acc[:, U : U + 1],
                    scalar=acc[:, U + 1 : U + 2],
                    in1=wmax[:, :],
                    op0=ADD,
                    op1=MULT,
                )
                wss[s] = ws

            def score_mm(s):
                ps = psaux.tile([1, 1], F32, tag="sc", name=f"sc{s}")
                nc.tensor.matmul(ps[:, :], ones16c, wss[s][:, :])
                scs[s] = ps

            def scale_out(s):
                nc.vector.tensor_scalar_mul(
                    out=out_sb[0:1, s : s + 1], in0=scs[s][:, :], scalar1=INV_HW
                )

            # ---- emission order (engine FIFOs follow program order) ----
            st_mms(0, 0, NST)          # PE, runs as soon as gtd0 lands
            st_reduce(0)               # V
            st16_mm(0)                 # PE (tiny fp32)
            den_r(0)                   # V, after gpw0
            inters_s_mms(0, 0, NCH, with_s=False)   # PE, after gpw0
            # S0 in a separate loop so inters0 MMs are not gated on r0
            ps_s0 = get_ps(0)[1]
            for c in range(NCH):
                nc.tensor.matmul(
                    ps_s0[:, :],
                    gpws[0][:, c * PART : (c + 1) * PART],
                    r2s[0][:, c * J * 2 : (c + 1) * J * 2],
                    start=(c == 0),
                    stop=(c == NCH - 1),
                )
            extM0, extS0 = extract_copies(0)        # Scalar
            pa0, pb0 = selectors(0, extM0, extS0)   # PE
            epilogue_v(0, pa0, pb0)                 # V

            den_r(1)                   # V, after gpw1
            # tail: st1 + inters1 + S1 interleaved, gated by gtd1 chunks
            stq = [(0, 1), (1, 3), (3, 5), (5, 7)]  # st groups ending in quarter
            for q in range(NQ):
                st_mms(1, *stq[q])
                inters_s_mms(1, QC[q][0], QC[q][1], with_s=True)
            extM1, extS1 = extract_copies(1)
            pa1, pb1 = selectors(1, extM1, extS1)
            st_reduce(1)
            st16_mm(1)
            epilogue_v(1, pa1, pb1)
            score_mm(0)
            score_mm(1)
            scale_out(0)
            scale_out(1)

            nc.sync.dma_start(out=y[:, :], in_=out_sb[:, :])

    _split_multi_waits(nc)
    return nc


_NC = None


def _get_nc():
    global _NC
    if _NC is None:
        _NC = _build()
    return _NC


def make_in_maps(groups_pred: np.ndarray, groups_true: np.ndarray):
    gp = np.asarray(groups_pred, dtype=np.float32).reshape(N, P, PART, JW)
    gt = np.asarray(groups_true, dtype=np.float32).reshape(N, T, PART, JW)
    # weight layout: (n, part, j, p)
    gpw = np.ascontiguousarray(gp.transpose(0, 2, 3, 1)).reshape(
        NCORES, SPC, PART, JW, P
    )
    gtd = np.empty((N, PART, JW, U), dtype=np.float32)
    gtd[..., :T] = gt.transpose(0, 2, 3, 1)
    gtd[..., T] = 1.0
    gtd = gtd.reshape(NCORES, SPC, PART, JW, U)
    cab = np.zeros((PART, PART + 1), dtype=ml_dtypes.bfloat16)
    cab[:, :PART] = np.eye(PART, dtype=np.float32)
    cab[:, PART] = 1.0
    aux = np.ones((16, 17), dtype=np.float32)
    return [
        {"gpw": gpw[c], "gtd": gtd[c], "cab": cab, "aux": aux}
        for c in range(NCORES)
    ]


def kernel(groups_pred: np.ndarray, groups_true: np.ndarray) -> np.ndarray:
    assert groups_pred.shape == (N, P, H, W)
    assert groups_true.shape == (N, T, H, W)
    in_maps = make_in_maps(groups_pred, groups_true)
    res = run_bass_kernel_spmd(_get_nc(), in_maps, core_ids=list(range(NCORES)))
    out = np.empty((N,), dtype=np.float32)
    for c in range(NCORES):
        out[c * SPC : (c + 1) * SPC] = res.results[c]["y"][0]
    return out
